# revision 1
# baseline (speedup 1.0000x reference)
"""Multi-head causal attention (B=2, S=2048, E=1024, H=16, D=64) on 8 TRN2
NeuronCores.

Sharding (data + tensor parallel, Megatron-style):
  core c -> batch b = c // 4, head group g = c % 4 (4 heads, e' = 256 cols).
  Wq/Wk/Wv column-sharded ([256, 1024] slices), Wo row-sharded
  ([1024, 256] slice); each core produces a partial output [2048, 1024]
  which the host sums per batch group (the Megatron all-reduce) and adds bo.

Per-core device kernel (all matmuls in float32r = fp32 data, FP22 multiply):
  K^T = Wk_l x_k^T + bk  [256, 2048]   (e' on partitions -> heads x 64)
  Q^T = Wq_l x_q^T + bq  [256, 2048]
  V'  = [x_v Wv_l^T + bv | 1]  [2048, 4*65]  (ones col for softmax denom)
  per (q-tile 512, head pair): S^T tiles [128 k, 512 q] via PE (2 heads
  packed in PE row groups 0-63/64-127), causal mask add (DVE), exp via ACT
  (scale=1/8 folded), acc += V'^T @ exp accumulated in PSUM [65, 512];
  row 64 = softmax denominator; normalize via DVE reciprocal + GpSimd
  partition broadcast; O-projection from vals^T with Wo_l^T.
"""
import sys
import os

sys.path.insert(0, "/opt/trn_rl_repo")

import numpy as np
from contextlib import ExitStack

import concourse.bass as bass  # noqa: E402
import concourse.mybir as mybir  # noqa: E402
import concourse.tile as tile  # noqa: E402
from concourse import bacc, bass_utils  # noqa: E402

bass_utils.upload_artifacts = lambda d: f"local:{d}"

B, S, E, H, D = 2, 2048, 1024, 16, 64
NCORES = 8
EL = 256  # e' columns per core (4 heads)
F32 = mybir.dt.float32
F32R = mybir.dt.float32r
AF = mybir.ActivationFunctionType

_CACHE = {}


def _build():
    nc = bacc.Bacc("TRN2", target_bir_lowering=False, debug=False)

    xq_d = nc.dram_tensor("xqT", [E, S], F32R, kind="ExternalInput")
    xk_d = nc.dram_tensor("xkT", [E, S], F32R, kind="ExternalInput")
    xv_d = nc.dram_tensor("xvT", [E, S], F32R, kind="ExternalInput")
    wq_d = nc.dram_tensor("wqT", [E, EL], F32R, kind="ExternalInput")
    wk_d = nc.dram_tensor("wkT", [E, EL], F32R, kind="ExternalInput")
    wv_d = nc.dram_tensor("wvT", [E, EL], F32R, kind="ExternalInput")
    wo_d = nc.dram_tensor("woT", [EL, E], F32R, kind="ExternalInput")
    bq_d = nc.dram_tensor("bq", [EL], F32, kind="ExternalInput")
    bk_d = nc.dram_tensor("bk", [EL], F32, kind="ExternalInput")
    bv_d = nc.dram_tensor("bv", [EL], F32R, kind="ExternalInput")
    ones_d = nc.dram_tensor("ones1", [1, 128], F32R, kind="ExternalInput")
    vones_d = nc.dram_tensor("vones", [128, 16, 4, 1], F32R, kind="ExternalInput")
    mask_d = nc.dram_tensor("masks", [4, 128, 512], F32, kind="ExternalInput")
    out_d = nc.dram_tensor("out", [S, E], F32, kind="ExternalOutput")

    with tile.TileContext(nc) as tc, ExitStack() as ctx:
        cpool = ctx.enter_context(tc.tile_pool(name="const", bufs=1))
        xpool = ctx.enter_context(tc.tile_pool(name="xs", bufs=10))
        pp = ctx.enter_context(tc.tile_pool(name="pp", bufs=2, space="PSUM"))
        lgp = ctx.enter_context(tc.tile_pool(name="lgp", bufs=2, space="PSUM"))
        accp = ctx.enter_context(tc.tile_pool(name="accp", bufs=2, space="PSUM"))
        expp = ctx.enter_context(tc.tile_pool(name="expp", bufs=3))
        opool = ctx.enter_context(tc.tile_pool(name="op", bufs=3))
        smp = ctx.enter_context(tc.tile_pool(name="smp", bufs=4))

        # ---- constants ----
        wq = cpool.tile([128, 8, EL], F32R, tag="wq")
        nc.sync.dma_start(wq[:], wq_d.ap().rearrange("(k p) m -> p k m", p=128))
        wk = cpool.tile([128, 8, EL], F32R, tag="wk")
        nc.sync.dma_start(wk[:], wk_d.ap().rearrange("(k p) m -> p k m", p=128))
        wv = cpool.tile([128, 8, EL], F32R, tag="wv")
        nc.sync.dma_start(wv[:], wv_d.ap().rearrange("(k p) m -> p k m", p=128))
        wo = cpool.tile([128, 2, E], F32R, tag="wo")
        nc.sync.dma_start(wo[:], wo_d.ap().rearrange("(c p) m -> p c m", p=128))
        mk = cpool.tile([128, 4, 512], F32, tag="mk")
        nc.sync.dma_start(mk[:], mask_d.ap().rearrange("k p m -> p k m"))
        bqt = cpool.tile([128, 2], F32, tag="bqt")
        nc.sync.dma_start(bqt[:], bq_d.ap().rearrange("(c p) -> p c", p=128))
        bkt = cpool.tile([128, 2], F32, tag="bkt")
        nc.sync.dma_start(bkt[:], bk_d.ap().rearrange("(c p) -> p c", p=128))
        bvt = cpool.tile([1, EL], F32R, tag="bvt")
        nc.sync.dma_start(bvt[:], bv_d.ap().rearrange("(p m) -> p m", p=1))
        ones1 = cpool.tile([1, 128], F32R, tag="ones1")
        nc.sync.dma_start(ones1[:], ones_d.ap())

        KT = cpool.tile([128, 2, S], F32R, tag="KT")
        QT = cpool.tile([128, 2, S], F32R, tag="QT")
        VP = cpool.tile([128, 16, 4 * 65], F32R, tag="VP")
        valsT = cpool.tile([128, 2, S], F32R, tag="valsT")

        # ones columns of V' (col 64 of each 65-block)
        nc.sync.dma_start(
            VP[:].rearrange("p k (h x) -> p k h x", h=4)[:, :, :, 64:65],
            vones_d.ap(),
        )

        # ---- transposed projections: out^T[e', t] = W x^T + b ----
        def proj_T(x_d, w_t, bias_t, out_t, pfx):
            for th in range(2):
                xc = []
                for k in range(8):
                    t = xpool.tile([128, 1024], F32R, tag="xc",
                                   name=f"{pfx}x{th}_{k}")
                    nc.sync.dma_start(
                        t[:], x_d.ap()[k * 128:(k + 1) * 128,
                                       th * 1024:(th + 1) * 1024])
                    xc.append(t)
                for c in range(2):
                    for t2 in range(2):
                        ps = pp.tile([128, 512], F32, tag="pp",
                                     name=f"{pfx}ps{th}_{c}_{t2}")
                        for k in range(8):
                            nc.tensor.matmul(
                                ps[:],
                                lhsT=w_t[:, k, c * 128:(c + 1) * 128],
                                rhs=xc[k][:, t2 * 512:(t2 + 1) * 512],
                                start=(k == 0), stop=(k == 7))
                        tt = th * 2 + t2
                        nc.vector.tensor_scalar_add(
                            out_t[:, c, tt * 512:(tt + 1) * 512],
                            ps[:], bias_t[:, c:c + 1])

        proj_T(xk_d, wk, bkt, KT, "k")

        # ---- V projection (natural layout, bias folded via ones row) ----
        for th in range(2):
            xc = []
            for k in range(8):
                t = xpool.tile([128, 1024], F32R, tag="xc", name=f"vx{th}_{k}")
                nc.sync.dma_start(
                    t[:], xv_d.ap()[k * 128:(k + 1) * 128,
                                    th * 1024:(th + 1) * 1024])
                xc.append(t)
            for t3 in range(8):
                tt = th * 8 + t3
                ps = pp.tile([128, EL], F32, tag="pp", name=f"vps{tt}")
                for k in range(8):
                    nc.tensor.matmul(
                        ps[:],
                        lhsT=xc[k][:, t3 * 128:(t3 + 1) * 128],
                        rhs=wv[:, k, :],
                        start=(k == 0), stop=False)
                nc.tensor.matmul(ps[:], lhsT=ones1[:], rhs=bvt[:],
                                 start=False, stop=True)
                nc.vector.tensor_copy(
                    VP[:, tt, :].rearrange("p (h x) -> p h x", h=4)[:, :, 0:64],
                    ps[:].rearrange("p (h x) -> p h x", h=4))

        proj_T(xq_d, wq, bqt, QT, "q")

        # ---- attention ----
        for qt in range(4):
            nkt = 4 * qt + 4
            for c in range(2):
                accs = [accp.tile([65, 512], F32, tag="acc",
                                  name=f"acc{qt}_{c}_{hh}") for hh in range(2)]
                for kt in range(nkt):
                    lg = lgp.tile([128, 2, 512], F32, tag="lg",
                                  name=f"lg{qt}_{c}_{kt}")
                    for hh in range(2):
                        nc.tensor.matmul(
                            lg[:, hh, :],
                            lhsT=KT[hh * 64:(hh + 1) * 64, c,
                                    kt * 128:(kt + 1) * 128],
                            rhs=QT[hh * 64:(hh + 1) * 64, c,
                                   qt * 512:(qt + 1) * 512],
                            start=True, stop=True)
                    dd = kt * 128 - qt * 512
                    if dd >= 0:
                        for hh in range(2):
                            nc.vector.tensor_add(lg[:, hh, :], lg[:, hh, :],
                                                 mk[:, dd // 128, :])
                    ex = expp.tile([128, 2, 512], F32R, tag="ex",
                                   name=f"ex{qt}_{c}_{kt}")
                    nc.scalar.activation(ex[:, :, :], lg[:, :, :], AF.Exp,
                                         scale=0.125)
                    for hh in range(2):
                        h = 2 * c + hh
                        nc.tensor.matmul(
                            accs[hh][:],
                            lhsT=VP[:, kt, h * 65:(h + 1) * 65],
                            rhs=ex[:, hh, :],
                            start=(kt == 0), stop=(kt == nkt - 1),
                            skip_group_check=True)
                for hh in range(2):
                    sv = smp.tile([65, 512], F32, tag="sv",
                                  name=f"sv{qt}_{c}_{hh}")
                    nc.vector.tensor_copy(sv[:], accs[hh][:])
                    rc = smp.tile([1, 512], F32, tag="rc",
                                  name=f"rc{qt}_{c}_{hh}")
                    nc.vector.reciprocal(rc[:], sv[64:65, :])
                    bc = smp.tile([64, 512], F32, tag="bc",
                                  name=f"bc{qt}_{c}_{hh}")
                    nc.gpsimd.partition_broadcast(bc[:], rc[:])
                    nc.vector.tensor_mul(
                        valsT[hh * 64:(hh + 1) * 64, c,
                              qt * 512:(qt + 1) * 512],
                        sv[0:64, :], bc[:])

        # ---- O projection (partial; host sums over head groups + bo) ----
        for tt in range(16):
            ot = opool.tile([128, E], F32, tag="ot", name=f"ot{tt}")
            for eo in range(2):
                ps = pp.tile([128, 512], F32, tag="pp", name=f"ops{tt}_{eo}")
                for c in range(2):
                    nc.tensor.matmul(
                        ps[:],
                        lhsT=valsT[:, c, tt * 128:(tt + 1) * 128],
                        rhs=wo[:, c, eo * 512:(eo + 1) * 512],
                        start=(c == 0), stop=(c == 1))
                nc.vector.tensor_copy(ot[:, eo * 512:(eo + 1) * 512], ps[:])
            nc.sync.dma_start(out_d.ap()[tt * 128:(tt + 1) * 128, :], ot[:])

    nc.compile()
    return nc


def get_nc():
    if "nc" not in _CACHE:
        _CACHE["nc"] = _build()
    return _CACHE["nc"]


def _masks():
    i = np.arange(128)[:, None]
    j = np.arange(512)[None, :]
    m = np.zeros((4, 128, 512), dtype=np.float32)
    for di in range(4):
        m[di] = np.where(i + di * 128 <= j, 0.0, -1e9)
    return m


def make_in_maps(query, key, value, Wq, bq, Wk, bk, Wv, bv, Wo, bo):
    query = np.asarray(query, np.float32)
    key = np.asarray(key, np.float32)
    value = np.asarray(value, np.float32)
    Wq, Wk, Wv, Wo = (np.asarray(a, np.float32) for a in (Wq, Wk, Wv, Wo))
    bq, bk, bv = (np.asarray(a, np.float32) for a in (bq, bk, bv))
    masks = _masks()
    ones1 = np.ones((1, 128), np.float32)
    vones = np.ones((128, 16, 4, 1), np.float32)
    in_maps = []
    for c in range(NCORES):
        b, g = divmod(c, 4)
        sl = slice(g * EL, (g + 1) * EL)
        in_maps.append({
            "xqT": np.ascontiguousarray(query[b].T),
            "xkT": np.ascontiguousarray(key[b].T),
            "xvT": np.ascontiguousarray(value[b].T),
            "wqT": np.ascontiguousarray(Wq[sl, :].T),
            "wkT": np.ascontiguousarray(Wk[sl, :].T),
            "wvT": np.ascontiguousarray(Wv[sl, :].T),
            "woT": np.ascontiguousarray(Wo[:, sl].T),
            "bq": np.ascontiguousarray(bq[sl]),
            "bk": np.ascontiguousarray(bk[sl]),
            "bv": np.ascontiguousarray(bv[sl]),
            "ones1": ones1,
            "vones": vones,
            "masks": masks,
        })
    return in_maps


def run(inputs, trace=False, tmpdir=None):
    """Run on 8 cores; returns (full_output, BassKernelResults)."""
    nc = get_nc()
    in_maps = make_in_maps(**inputs)
    res = bass_utils.run_bass_kernel_spmd(
        nc, in_maps, list(range(NCORES)), trace=trace, tmpdir=tmpdir)
    bo = np.asarray(inputs["bo"], np.float32)
    out = np.zeros((B, S, E), np.float32)
    for c in range(NCORES):
        out[c // 4] += res.results[c]["out"]
    out += bo[None, None, :]
    return out, res


def kernel(**inputs):
    out, _ = run(inputs)
    return out


# revision 4
# speedup vs baseline: 1.4027x; 1.4027x over previous
"""Multi-head causal attention (B=2, S=2048, E=1024, H=16, D=64) on 8 TRN2
NeuronCores.

Sharding (data + tensor parallel, Megatron-style):
  core c -> batch b = c // 4, head group g = c % 4 (4 heads, e' = 256 cols).
  Wq/Wk/Wv column-sharded ([256, 1024] slices), Wo row-sharded
  ([1024, 256] slice); each core produces a partial output [2048, 1024]
  which the host sums per batch group (the Megatron all-reduce) and adds bo.

Per-core device kernel (matmul operands fp16, accumulate fp32 in PSUM):
  K^T = Wk_l x_k^T + bk  [256, 2048]   (e' on partitions -> heads x 64)
  Q^T = Wq_l x_q^T + bq  [256, 2048]
  V'  = [x_v Wv_l^T + bv | 1]  [2048, 4*65]  (ones col -> softmax denom)
  attention in S^T orientation: per (q-tile 512, head pair), S^T tiles
  [128 k, 512 q] via PE (2 heads packed in PE row groups 0-63/64-127),
  exp via ACT (1/8 scale folded), multiplicative causal mask on the exp
  (diagonal tiles only), acc += V'^T @ P^T accumulated in PSUM [65, 512];
  row 64 = softmax denominator; normalize via DVE reciprocal_approx_fast +
  GpSimd partition broadcast; O-projection from vals^T with Wo_l^T,
  interleaved per q-tile. The k-loop is software-pipelined: attnV(kt-1) is
  emitted after logits/exp(kt) so ACT and PE overlap.
"""
import sys
import os

sys.path.insert(0, "/opt/trn_rl_repo")

import numpy as np
from contextlib import ExitStack

import concourse.bass as bass  # noqa: E402
import concourse.mybir as mybir  # noqa: E402
import concourse.tile as tile  # noqa: E402
from concourse import bacc, bass_utils  # noqa: E402

bass_utils.upload_artifacts = lambda d: f"local:{d}"

B, S, E, H, D = 2, 2048, 1024, 16, 64
NCORES = 8
EL = 256  # e' columns per core (4 heads)
F32 = mybir.dt.float32
F16 = mybir.dt.float16
AF = mybir.ActivationFunctionType
NP16 = np.float16

_CACHE = {}


def _build():
    nc = bacc.Bacc("TRN2", target_bir_lowering=False, debug=False)

    xq_d = nc.dram_tensor("xqT", [E, S], F16, kind="ExternalInput")
    xk_d = nc.dram_tensor("xkT", [E, S], F16, kind="ExternalInput")
    xv_d = nc.dram_tensor("xvT", [E, S], F16, kind="ExternalInput")
    wq_d = nc.dram_tensor("wqT", [E, EL], F16, kind="ExternalInput")
    wk_d = nc.dram_tensor("wkT", [E, EL], F16, kind="ExternalInput")
    wv_d = nc.dram_tensor("wvT", [E, EL], F16, kind="ExternalInput")
    wo_d = nc.dram_tensor("woT", [EL, E], F16, kind="ExternalInput")
    bq_d = nc.dram_tensor("bq", [EL], F32, kind="ExternalInput")
    bk_d = nc.dram_tensor("bk", [EL], F32, kind="ExternalInput")
    bv_d = nc.dram_tensor("bv", [EL], F32, kind="ExternalInput")
    vones_d = nc.dram_tensor("vones", [128, 16, 4, 1], F16, kind="ExternalInput")
    mask_d = nc.dram_tensor("masks", [4, 128, 512], F16, kind="ExternalInput")
    out_d = nc.dram_tensor("out", [S, E], F32, kind="ExternalOutput")

    with tile.TileContext(nc) as tc, ExitStack() as ctx:
        cpool = ctx.enter_context(tc.tile_pool(name="const", bufs=1))
        pp = ctx.enter_context(tc.tile_pool(name="pp", bufs=2, space="PSUM"))
        lgp = ctx.enter_context(tc.tile_pool(name="lgp", bufs=2, space="PSUM"))
        accp = ctx.enter_context(tc.tile_pool(name="accp", bufs=2, space="PSUM"))
        expp = ctx.enter_context(tc.tile_pool(name="expp", bufs=4))
        opool = ctx.enter_context(tc.tile_pool(name="op", bufs=4))
        smp = ctx.enter_context(tc.tile_pool(name="smp", bufs=4))

        # ---- constants ----
        wq = cpool.tile([128, 8, EL], F16, tag="wq")
        nc.sync.dma_start(wq[:], wq_d.ap().rearrange("(k p) m -> p k m", p=128))
        wk = cpool.tile([128, 8, EL], F16, tag="wk")
        nc.sync.dma_start(wk[:], wk_d.ap().rearrange("(k p) m -> p k m", p=128))
        wv = cpool.tile([128, 8, EL], F16, tag="wv")
        nc.sync.dma_start(wv[:], wv_d.ap().rearrange("(k p) m -> p k m", p=128))
        wo = cpool.tile([128, 2, E], F16, tag="wo")
        nc.sync.dma_start(wo[:], wo_d.ap().rearrange("(c p) m -> p c m", p=128))
        mk = cpool.tile([128, 4, 512], F16, tag="mk")
        nc.sync.dma_start(mk[:], mask_d.ap().rearrange("k p m -> p k m"))
        bqt = cpool.tile([128, 2], F32, tag="bqt")
        nc.sync.dma_start(bqt[:], bq_d.ap().rearrange("(c p) -> p c", p=128))
        bkt = cpool.tile([128, 2], F32, tag="bkt")
        nc.sync.dma_start(bkt[:], bk_d.ap().rearrange("(c p) -> p c", p=128))
        bvr = cpool.tile([1, EL], F32, tag="bvr")
        nc.sync.dma_start(bvr[:], bv_d.ap().rearrange("(p m) -> p m", p=1))
        bvb = cpool.tile([128, EL], F32, tag="bvb")
        nc.gpsimd.partition_broadcast(bvb[:], bvr[:])

        # full x^T tensors (fp16): [128, 8 E-chunks, 2048]
        xq = cpool.tile([128, 8, S], F16, tag="xq")
        nc.sync.dma_start(xq[:], xq_d.ap().rearrange("(k p) m -> p k m", p=128))
        xk = cpool.tile([128, 8, S], F16, tag="xk")
        nc.sync.dma_start(xk[:], xk_d.ap().rearrange("(k p) m -> p k m", p=128))
        xv = cpool.tile([128, 8, S], F16, tag="xv")
        nc.sync.dma_start(xv[:], xv_d.ap().rearrange("(k p) m -> p k m", p=128))

        KT = cpool.tile([128, 2, S], F16, tag="KT")
        QT = cpool.tile([128, 2, S], F16, tag="QT")
        VP = cpool.tile([128, 16, 4 * 66], F16, tag="VP")  # 66: keep 4B-aligned fp16 offsets
        valsT = cpool.tile([128, 2, S], F16, tag="valsT")

        # ones columns of V' (col 64 of each 65-block)
        nc.sync.dma_start(
            VP[:].rearrange("p k (h x) -> p k h x", h=4)[:, :, :, 64:65],
            vones_d.ap(),
        )

        # ---- transposed projections: out^T[e', t] = W x^T + b ----
        def proj_T(x_t, w_t, bias_t, out_t, pfx):
            for c in range(2):
                for tt in range(4):
                    ps = pp.tile([128, 512], F32, tag="pp",
                                 name=f"{pfx}ps{c}_{tt}")
                    for k in range(8):
                        nc.tensor.matmul(
                            ps[:],
                            lhsT=w_t[:, k, c * 128:(c + 1) * 128],
                            rhs=x_t[:, k, tt * 512:(tt + 1) * 512],
                            start=(k == 0), stop=(k == 7))
                    nc.vector.tensor_scalar_add(
                        out_t[:, c, tt * 512:(tt + 1) * 512],
                        ps[:], bias_t[:, c:c + 1])

        proj_T(xk, wk, bkt, KT, "k")

        # ---- V projection (natural layout) ----
        for t3 in range(16):
            ps = pp.tile([128, EL], F32, tag="pp", name=f"vps{t3}")
            for k in range(8):
                nc.tensor.matmul(
                    ps[:],
                    lhsT=xv[:, k, t3 * 128:(t3 + 1) * 128],
                    rhs=wv[:, k, :],
                    start=(k == 0), stop=(k == 7))
            nc.vector.tensor_add(
                VP[:, t3, :].rearrange("p (h x) -> p h x", h=4)[:, :, 0:64],
                ps[:].rearrange("p (h x) -> p h x", h=4),
                bvb[:].rearrange("p (h x) -> p h x", h=4))

        proj_T(xq, wq, bqt, QT, "q")

        # ---- attention + O-projection, interleaved per q-tile ----
        for qt in range(4):
            nkt = 4 * qt + 4
            for c in range(2):
                accs = [accp.tile([65, 512], F32, tag="acc",
                                  name=f"acc{qt}_{c}_{hh}") for hh in range(2)]
                exs = {}

                def attn_v(kt):
                    st = (kt == 0)
                    sp = (kt == nkt - 1)
                    for hh in range(2):
                        h = 2 * c + hh
                        nc.tensor.matmul(
                            accs[hh][:],
                            lhsT=VP[:, kt, h * 66:h * 66 + 65],
                            rhs=exs[kt][:, hh, :],
                            start=st, stop=sp, skip_group_check=True)

                for kt in range(nkt):
                    lg = lgp.tile([128, 2, 512], F32, tag="lg",
                                  name=f"lg{qt}_{c}_{kt}")
                    for hh in range(2):
                        nc.tensor.matmul(
                            lg[:, hh, :],
                            lhsT=KT[hh * 64:(hh + 1) * 64, c,
                                    kt * 128:(kt + 1) * 128],
                            rhs=QT[hh * 64:(hh + 1) * 64, c,
                                   qt * 512:(qt + 1) * 512],
                            start=True, stop=True)
                    ex = expp.tile([128, 2, 512], F16, tag="ex",
                                   name=f"ex{qt}_{c}_{kt}")
                    nc.scalar.activation(ex[:, :, :], lg[:, :, :], AF.Exp,
                                         scale=0.125)
                    dd = kt * 128 - qt * 512
                    if dd >= 0:  # diagonal tile: multiplicative causal mask
                        for hh in range(2):
                            nc.vector.tensor_mul(ex[:, hh, :], ex[:, hh, :],
                                                 mk[:, dd // 128, :])
                    exs[kt] = ex
                    if kt >= 1:
                        attn_v(kt - 1)  # sw-pipeline: overlap exp(kt) on ACT
                attn_v(nkt - 1)

                for hh in range(2):
                    sv = smp.tile([65, 512], F32, tag="sv",
                                  name=f"sv{qt}_{c}_{hh}")
                    nc.vector.tensor_copy(sv[:], accs[hh][:])
                    rc = smp.tile([1, 512], F32, tag="rc",
                                  name=f"rc{qt}_{c}_{hh}")
                    nc.vector.reciprocal(rc[:], sv[64:65, :])
                    bc = smp.tile([64, 512], F32, tag="bc",
                                  name=f"bc{qt}_{c}_{hh}")
                    nc.gpsimd.partition_broadcast(bc[:], rc[:])
                    nc.vector.tensor_mul(
                        valsT[hh * 64:(hh + 1) * 64, c,
                              qt * 512:(qt + 1) * 512],
                        sv[0:64, :], bc[:])

            # O-projection for this q-tile's four 128-row chunks
            for tt in range(4 * qt, 4 * qt + 4):
                ot = opool.tile([128, E], F32, tag="ot", name=f"ot{tt}")
                for eo in range(2):
                    ps = pp.tile([128, 512], F32, tag="pp",
                                 name=f"ops{tt}_{eo}")
                    for c in range(2):
                        nc.tensor.matmul(
                            ps[:],
                            lhsT=valsT[:, c, tt * 128:(tt + 1) * 128],
                            rhs=wo[:, c, eo * 512:(eo + 1) * 512],
                            start=(c == 0), stop=(c == 1))
                    nc.vector.tensor_copy(ot[:, eo * 512:(eo + 1) * 512],
                                          ps[:])
                nc.sync.dma_start(out_d.ap()[tt * 128:(tt + 1) * 128, :],
                                  ot[:])

    nc.compile()
    return nc


def get_nc():
    if "nc" not in _CACHE:
        _CACHE["nc"] = _build()
    return _CACHE["nc"]


def _masks():
    i = np.arange(128)[:, None]
    j = np.arange(512)[None, :]
    m = np.zeros((4, 128, 512), dtype=NP16)
    for di in range(4):
        m[di] = (i + di * 128 <= j).astype(NP16)
    return m


def make_in_maps(query, key, value, Wq, bq, Wk, bk, Wv, bv, Wo, bo):
    query = np.asarray(query, np.float32)
    key = np.asarray(key, np.float32)
    value = np.asarray(value, np.float32)
    Wq, Wk, Wv, Wo = (np.asarray(a, np.float32) for a in (Wq, Wk, Wv, Wo))
    bq, bk, bv = (np.asarray(a, np.float32) for a in (bq, bk, bv))
    masks = _masks()
    vones = np.ones((128, 16, 4, 1), NP16)
    in_maps = []
    for c in range(NCORES):
        b, g = divmod(c, 4)
        sl = slice(g * EL, (g + 1) * EL)
        in_maps.append({
            "xqT": np.ascontiguousarray(query[b].T).astype(NP16),
            "xkT": np.ascontiguousarray(key[b].T).astype(NP16),
            "xvT": np.ascontiguousarray(value[b].T).astype(NP16),
            "wqT": np.ascontiguousarray(Wq[sl, :].T).astype(NP16),
            "wkT": np.ascontiguousarray(Wk[sl, :].T).astype(NP16),
            "wvT": np.ascontiguousarray(Wv[sl, :].T).astype(NP16),
            "woT": np.ascontiguousarray(Wo[:, sl].T).astype(NP16),
            "bq": np.ascontiguousarray(bq[sl]),
            "bk": np.ascontiguousarray(bk[sl]),
            "bv": np.ascontiguousarray(bv[sl]),
            "vones": vones,
            "masks": masks,
        })
    return in_maps


def run(inputs, trace=False, tmpdir=None):
    """Run on 8 cores; returns (full_output, BassKernelResults)."""
    nc = get_nc()
    in_maps = make_in_maps(**inputs)
    res = bass_utils.run_bass_kernel_spmd(
        nc, in_maps, list(range(NCORES)), trace=trace, tmpdir=tmpdir)
    bo = np.asarray(inputs["bo"], np.float32)
    out = np.zeros((B, S, E), np.float32)
    for c in range(NCORES):
        out[c // 4] += res.results[c]["out"]
    out += bo[None, None, :]
    return out, res


def kernel(**inputs):
    out, _ = run(inputs)
    return out
